# revision 26
# baseline (speedup 1.0000x reference)
"""GroupedQueryAttention (B=1, T=2048, D=4096, 32 q-heads / 8 kv-heads, hd=128)
on 8 trn2 NeuronCores.

Sharding: kv-head parallel — core c owns kv head c and its 4 query heads.
Mixed precision: fp16 for x/weights/q/k (projections, rope, score matmuls),
bf16 for exp/l/PV (bf16 has fp32 range — scaled scores reach ~50, exp(50)
overflows fp16), fp16 again for the normalized y and the column-parallel wo
matmul after a chunked AllGather. All matmuls run at 1 cycle/row (vs 4 for
fp32). Softmax without max-subtraction (bf16 exp can't overflow here).

Pipeline per 512-token chunk qc:
  QKV sweeps (x chunk in SBUF halves, one PSUM accumulator per sweep,
  order v,k,q0..q3 so the attention dependencies finish earliest)
  -> rope -> causal attention in transposed [k, q] layout (exp fused over
  1024-wide score pairs; the softmax epilogue -- denominator matmuls,
  reciprocal, gpsimd partition-broadcast, normalize -- is deferred in two
  stages into the next strip so the PE never waits on it)
  -> AllGather of the y.T chunk (overlapped with later compute; a tiny
  warmup AllGather at kernel start absorbs the ncfw first-collective cost)
  -> column-parallel wo matmul two chunks later, with its y reads issued
  on the gpsimd/SWDGE queue right after the matching AllGather so the
  sync DMA ring never stalls on a collective.
The last chunk's AllGather is split by head pairs so it overlaps the last
two attention strips; wo for that chunk indexes wo_sb tiles to match the
rank-major order of the two half-gathers, and runs all first-half matmuls
before the second half-gather lands.
"""
import sys

sys.path.insert(0, "/opt/trn_rl_repo")

import numpy as np
import ml_dtypes

import concourse.bacc as bacc
import concourse.tile as tile
from concourse import mybir
from concourse.bass_utils import run_bass_kernel_spmd
from concourse.masks import make_identity

N_CORES = 8
T = 2048
DIM = 4096
HD = 128
NH = 32
NKV = 8
NREP = NH // NKV  # 4 query heads per core
CH = 512  # chunk length along T
NCHUNK = T // CH  # 4
NKT = DIM // 128  # 32 contraction tiles for the projections
F32 = mybir.dt.float32
F16 = mybir.dt.float16
BF16 = mybir.dt.bfloat16
SCALE = 1.0 / float(np.sqrt(HD))
BF16NP = ml_dtypes.bfloat16
LAST = NCHUNK - 1

_cached = {}


def _build_kernel():
    if "nc" in _cached:
        return _cached["nc"]

    nc = bacc.Bacc("TRN2", target_bir_lowering=False)

    xt = nc.dram_tensor(
        "xt", [NCHUNK, 2, 128, 16 * CH], F16, kind="ExternalInput"
    )
    cos2 = nc.dram_tensor("cos2", [128, T], F16, kind="ExternalInput")
    sin2 = nc.dram_tensor("sin2", [128, T], F16, kind="ExternalInput")
    masks = nc.dram_tensor("masks", [128, 4 * CH], BF16, kind="ExternalInput")
    # weight groups pre-transposed on host to [128, NKT*128] per group,
    # in sweep order v, q0..q3, k — each group load is one contiguous DMA
    wqkv = nc.dram_tensor("wqkv", [6, 128, NKT * 128], F16, kind="ExternalInput")
    woT = nc.dram_tensor("woT", [DIM, NREP * HD], F16, kind="ExternalInput")
    out = nc.dram_tensor("out", [T, NREP * HD], F16, kind="ExternalOutput")

    y_in = [
        nc.dram_tensor(f"y_in{qc}", [NREP * HD, CH], F16, kind="Internal")
        for qc in range(LAST)
    ]
    y_all = [
        nc.dram_tensor(
            f"y_all{qc}", [DIM, CH], F16, kind="Internal", addr_space="Shared"
        )
        for qc in range(LAST)
    ]
    # last chunk: two half-gathers (head pairs) so the collective overlaps
    # the tail attention strips
    wu_in = nc.dram_tensor("wu_in", [128, 2], F16, kind="Internal")
    wu_out = nc.dram_tensor(
        "wu_out", [128 * N_CORES, 2], F16, kind="Internal", addr_space="Shared"
    )
    y_in_l = [
        nc.dram_tensor(f"y_inl{i}", [2 * HD, CH], F16, kind="Internal")
        for i in range(2)
    ]
    y_all_l = [
        nc.dram_tensor(
            f"y_alll{i}", [DIM // 2, CH], F16, kind="Internal",
            addr_space="Shared",
        )
        for i in range(2)
    ]

    with tile.TileContext(nc) as tc:
        with (
            tc.tile_pool(name="consts", bufs=1) as consts,
            tc.tile_pool(name="weights", bufs=1) as weights,
            tc.tile_pool(name="xpool", bufs=3) as xpool,
            tc.tile_pool(name="acts", bufs=1) as acts,
            tc.tile_pool(name="qpool", bufs=1) as qpool,
            tc.tile_pool(name="work", bufs=2) as work,
            tc.tile_pool(name="expp", bufs=3) as expp,
            tc.tile_pool(name="outp", bufs=2) as outp,
            tc.tile_pool(name="ypool", bufs=3) as ypool,
            tc.tile_pool(name="pp", bufs=4, space="PSUM") as pp,
            tc.tile_pool(name="sp", bufs=2, space="PSUM") as sp,
        ):
            def ag(ins_t, outs_t):
                nc.gpsimd.collective_compute(
                    "AllGather",
                    mybir.AluOpType.bypass,
                    ins=[ins_t[:, :]],
                    outs=[outs_t[:, :]],
                    replica_groups=[list(range(N_CORES))],
                )

            def emit_x_load(qc):
                xh = []
                for i in range(2):
                    x_sb = xpool.tile(
                        [128, 16, CH], F16, tag="xsb", name=f"x{qc}_{i}"
                    )
                    nc.sync.dma_start(
                        out=x_sb,
                        in_=xt[qc, i].rearrange("p (n m) -> p n m", m=CH),
                    )
                    xh.append((16 * i, 16, x_sb))
                return xh

            # ---------- startup loads, in first-consumption order ----------
            wqkv_sb = weights.tile([128, 6, NKT, 128], F16, tag="wqkv")
            wq_r = wqkv.rearrange("g p (n m) -> g p n m", m=128)
            nc.sync.dma_start(out=wqkv_sb[:, 0, 0:16], in_=wq_r[0][:, 0:16])
            x_first = emit_x_load(0)
            nc.sync.dma_start(out=wqkv_sb[:, 0, 16:32], in_=wq_r[0][:, 16:32])
            # tiny warmup collective: absorbs the first-AllGather penalty
            ag(wu_in, wu_out)
            for g in (5, 1, 2, 3, 4):  # k then q0..q3, matching sweep order
                nc.sync.dma_start(out=wqkv_sb[:, g], in_=wq_r[g])

            # preload the ACT exp table before attention needs it
            dumm = consts.tile([1, 8], F32, tag="dummy")
            nc.vector.memset(dumm, 0.0)
            nc.scalar.activation(dumm, dumm, mybir.ActivationFunctionType.Exp)

            cos_sb = consts.tile([128, T], F16, tag="cos")
            nc.sync.dma_start(out=cos_sb, in_=cos2[:, :])
            sin_sb = consts.tile([128, T], F16, tag="sin")
            nc.sync.dma_start(out=sin_sb, in_=sin2[:, :])
            mask_sb = consts.tile([128, 4 * CH], BF16, tag="mask")
            nc.sync.dma_start(out=mask_sb, in_=masks[:, :])
            ones_col = consts.tile([128, 1], BF16, tag="onesc")
            nc.vector.memset(ones_col, 1.0)
            ident = consts.tile([128, 128], BF16, tag="ident")
            make_identity(nc, ident)

            wo_sb = weights.tile([128, NKT, NREP * HD], F16, tag="wo")
            nc.sync.dma_start(
                out=wo_sb, in_=woT.rearrange("(n p) m -> p n m", p=128)
            )

            # K^T and V tiles persist across the whole attention phase
            kT_sb = acts.tile([128, T], F16, tag="kt")
            vkd_sb = acts.tile([128, T // 128, HD], BF16, tag="vkd")

            def qkv_phase(qc, xh):
                cs = slice(CH * qc, CH * (qc + 1))

                def xtile(kt):
                    for k0, n, t in xh:
                        if k0 <= kt < k0 + n:
                            return t[:, kt - k0, :]

                # sweep order: v (group 0) first so the transposes at the
                # end have their input, then k (its rope feeds attention
                # first), then q0..q3
                v_ps = pp.tile([128, CH], F32, tag="bank", name=f"vps{qc}")
                for kt in range(NKT):
                    nc.tensor.matmul(
                        v_ps,
                        lhsT=wqkv_sb[:, 0, kt, :],
                        rhs=xtile(kt),
                        start=(kt == 0),
                        stop=(kt == NKT - 1),
                    )
                v_sb = work.tile([128, CH], BF16, tag="vsb")
                nc.scalar.copy(v_sb, v_ps)

                qT_sb = qpool.tile([128, NREP, CH], F16, tag="qt")
                for h in (NREP, 0, 1, 2, 3):
                    a_ps = pp.tile([128, CH], F32, tag="bank", name=f"aps{qc}_{h}")
                    g = 5 if h == NREP else 1 + h
                    for kt in range(NKT):
                        nc.tensor.matmul(
                            a_ps,
                            lhsT=wqkv_sb[:, g, kt, :],
                            rhs=xtile(kt),
                            start=(kt == 0),
                            stop=(kt == NKT - 1),
                        )
                    psb = work.tile([128, CH], F16, tag="psb")
                    nc.scalar.copy(psb, a_ps)
                    tmp = work.tile([128, CH], F16, tag="tmp")
                    nc.scalar.copy(tmp[0:64, :], a_ps[64:128, :])
                    nc.scalar.copy(tmp[64:128, :], a_ps[0:64, :])
                    dst = qT_sb[:, h, :] if h < NREP else kT_sb[:, cs]
                    nc.vector.tensor_mul(dst, psb, cos_sb[:, cs])
                    nc.vector.tensor_mul(tmp, tmp, sin_sb[:, cs])
                    nc.vector.tensor_add(dst, dst, tmp)

                # v computed in [hd, T] layout; transpose 128x128 blocks to [k, hd]
                for s in range(4):
                    vt_ps = pp.tile([128, 128], BF16, tag="bank", name=f"vt{qc}_{s}")
                    nc.tensor.transpose(vt_ps, v_sb[:, 128 * s:128 * (s + 1)], ident)
                    nc.vector.tensor_copy(vkd_sb[:, 4 * qc + s, :], vt_ps)
                return qT_sb

            def att_phase(qc, qT_sb):
                nkt = 4 * qc + 4  # causal: k tiles 0 .. 4*qc+3
                pairs = nkt // 2
                last = qc == LAST
                # previous strip's epilogue runs in two deferred stages so
                # the PE never waits on the exp/add tail or the reciprocal:
                #  A (at next strip's j==1): l-matmuls + reciprocal
                #  B (at next strip's j==2): gpsimd partition-broadcast of
                #    1/l, normalize, store
                # and the last PV pair is deferred past the next strip's
                # first score pair so its exp gets a head start
                pend_a = None
                pend_b = None
                pend_pv = []

                def flush_b(pend_b):
                    done_h = pend_b()
                    if last and done_h == 1:
                        ag(y_in_l[0], y_all_l[0])  # heads 0,1 gathered

                for h in range(NREP):
                    q_rhs = qT_sb[:, h, :]
                    yT_ps = pp.tile([128, CH], F32, tag="bank", name=f"yT{qc}_{h}")
                    l_acc = work.tile([128, 1024], BF16, tag="lacc")
                    es = []
                    for j in range(pairs):
                        s_ps = sp.tile([128, 1024], F32, tag="pair")
                        for d2 in range(2):
                            kt = 2 * j + d2
                            nc.tensor.matmul(
                                s_ps[:, 512 * d2:512 * (d2 + 1)],
                                lhsT=kT_sb[:, 128 * kt:128 * (kt + 1)],
                                rhs=q_rhs,
                                start=True,
                                stop=True,
                            )
                        e_sb = expp.tile([128, 1024], BF16, tag="exp")
                        nc.scalar.activation(
                            e_sb, s_ps, mybir.ActivationFunctionType.Exp,
                            scale=SCALE,
                        )
                        if j >= pairs - 2:  # diagonal pair: zero the k > q half
                            dd = j - (pairs - 2)
                            nc.vector.tensor_mul(
                                e_sb, e_sb, mask_sb[:, 1024 * dd:1024 * (dd + 1)]
                            )
                        if j == 0:
                            nc.vector.tensor_copy(l_acc, e_sb)
                        else:
                            nc.vector.tensor_add(l_acc, l_acc, e_sb)
                        es.append(e_sb)
                        if j <= 1 and pend_pv:
                            pend_pv.pop(0)()
                        if j == 1 and pend_a is not None:
                            pend_a()
                            pend_a = None
                        if j == 2 and pend_b is not None:
                            flush_b(pend_b)
                            pend_b = None
                        if j >= 2:  # PV lags two pairs so exp stays ahead
                            pj = j - 2
                            for d2 in range(2):
                                kt = 2 * pj + d2
                                nc.tensor.matmul(
                                    yT_ps,
                                    lhsT=vkd_sb[:, kt, :],
                                    rhs=es[pj][:, 512 * d2:512 * (d2 + 1)],
                                    start=(kt == 0),
                                    stop=False,
                                )
                    if pend_b is not None:  # pairs == 2 chunks
                        flush_b(pend_b)
                        pend_b = None

                    def pv_tail(pj, yT_ps=yT_ps, e_list=es):
                        def run():
                            for d2 in range(2):
                                kt = 2 * pj + d2
                                nc.tensor.matmul(
                                    yT_ps,
                                    lhsT=vkd_sb[:, kt, :],
                                    rhs=e_list[pj][:, 512 * d2:512 * (d2 + 1)],
                                    start=(kt == 0),
                                    stop=(kt == nkt - 1),
                                )
                        return run
                    pend_pv = [pv_tail(pairs - 2), pv_tail(pairs - 1)]
                    cell = {}

                    def epi_a(l_acc=l_acc, qc=qc, h=h, cell=cell):
                        # softmax denominator: sum l_acc halves over partitions
                        l_ps = pp.tile([1, CH], F32, tag="bank", name=f"l{qc}_{h}")
                        nc.tensor.matmul(
                            l_ps[0:1, :], lhsT=ones_col[:, 0:1],
                            rhs=l_acc[:, 0:512], start=True, stop=False,
                        )
                        nc.tensor.matmul(
                            l_ps[0:1, :], lhsT=ones_col[:, 0:1],
                            rhs=l_acc[:, 512:1024], start=False, stop=True,
                        )
                        r32 = work.tile([1, CH], F32, tag="r32")
                        nc.vector.reciprocal_approx_fast(r32, l_ps[0:1, :])
                        cell["r32"] = r32

                    def epi_b(yT_ps=yT_ps, h=h, qc=qc, last=last, cell=cell):
                        bc_sb = work.tile([128, CH], F32, tag="bc")
                        nc.gpsimd.partition_broadcast(bc_sb, cell["r32"][0:1, :])
                        yn_sb = work.tile([128, CH], F16, tag="yn")
                        nc.vector.tensor_mul(yn_sb, yT_ps, bc_sb)
                        if last:
                            dst = y_in_l[h // 2][128 * (h % 2):128 * (h % 2 + 1), :]
                        else:
                            dst = y_in[qc][128 * h:128 * (h + 1), :]
                        nc.sync.dma_start(out=dst, in_=yn_sb)
                        return h

                    pend_a = epi_a
                    pend_b = epi_b
                # y_in[qc] must be complete before its AllGather
                for f in pend_pv:
                    f()
                pend_a()
                flush_b(pend_b)
                if last:
                    ag(y_in_l[1], y_all_l[1])
                else:
                    ag(y_in[qc], y_all[qc])

            def wo_start(qc):
                # early y reads for the first two row-tiles, via the SWDGE
                # (gpsimd) path: on that queue they sit right after the
                # chunk's AllGather, so the sync ring never stalls on a
                # collective wait. [128, 16, 256] tiles give 512-byte runs
                y_r = y_all[qc].rearrange("(n p) m -> p n m", p=128)
                pre = []
                for i in range(2):  # contraction halves, t-cols 0:256
                    y_t = ypool.tile([128, 16, 256], F16, tag="yt")
                    nc.gpsimd.dma_start(
                        out=y_t, in_=y_r[:, 16 * i:16 * (i + 1), 0:256]
                    )
                    pre.append(y_t)
                return pre

            def wo_compute(qc, pre):
                y_r = y_all[qc].rearrange("(n p) m -> p n m", p=128)
                halves = {0: pre}
                for tt in range(CH // 128):
                    hh = tt // 2
                    if hh not in halves:
                        yts = []
                        for i in range(2):
                            y_t = ypool.tile([128, 16, 256], F16, tag="yt")
                            nc.sync.dma_start(
                                out=y_t,
                                in_=y_r[:, 16 * i:16 * (i + 1), 256:512],
                            )
                            yts.append(y_t)
                        halves[hh] = yts
                    yts = halves[hh]
                    sub = slice(128 * (tt % 2), 128 * (tt % 2) + 128)
                    o_ps = pp.tile([128, NREP * HD], F32, tag="bank")
                    for kt in range(NKT):
                        nc.tensor.matmul(
                            o_ps,
                            lhsT=yts[kt // 16][:, kt % 16, sub],
                            rhs=wo_sb[:, kt, :],
                            start=(kt == 0),
                            stop=(kt == NKT - 1),
                        )
                    o_sb = outp.tile([128, NREP * HD], F16, tag="osb")
                    nc.scalar.copy(o_sb, o_ps)
                    r0 = CH * qc + 128 * tt
                    nc.sync.dma_start(out=out[r0:r0 + 128, :], in_=o_sb)

            def wo_last():
                # chunk LAST: y arrives as two half-gathers, rank-major with
                # 2 head-tiles per rank; map each to its wo_sb k-tile. Run
                # all of the first half-gather's matmuls across the four row
                # tiles first so the second gather's latency hides behind them
                y_ra = y_all_l[0].rearrange("(n p) m -> p n m", p=128)
                ya = []
                for i in range(2):
                    y_t = ypool.tile([128, NKT // 2, 256], F16, tag="yt")
                    nc.sync.dma_start(
                        out=y_t, in_=y_ra[:, :, 256 * i:256 * (i + 1)]
                    )
                    ya.append(y_t)
                o_list = []
                for tt in range(CH // 128):
                    y_t = ya[tt // 2]
                    sub = slice(128 * (tt % 2), 128 * (tt % 2) + 128)
                    o_ps = pp.tile([128, NREP * HD], F32, tag="bank")
                    for n in range(NKT // 2):
                        kt = 4 * (n // 2) + (n % 2)
                        nc.tensor.matmul(
                            o_ps,
                            lhsT=y_t[:, n, sub],
                            rhs=wo_sb[:, kt, :],
                            start=(n == 0),
                            stop=False,
                        )
                    o_list.append(o_ps)
                y_rb = y_all_l[1].rearrange("(n p) m -> p n m", p=128)
                yb = []
                for i in range(2):
                    y_t = ypool.tile([128, NKT // 2, 256], F16, tag="yt")
                    nc.sync.dma_start(
                        out=y_t, in_=y_rb[:, :, 256 * i:256 * (i + 1)]
                    )
                    yb.append(y_t)
                for tt in range(CH // 128):
                    y_t = yb[tt // 2]
                    sub = slice(128 * (tt % 2), 128 * (tt % 2) + 128)
                    for n in range(NKT // 2):
                        kt = 4 * (n // 2) + 2 + (n % 2)
                        nc.tensor.matmul(
                            o_list[tt],
                            lhsT=y_t[:, n, sub],
                            rhs=wo_sb[:, kt, :],
                            start=False,
                            stop=(n == NKT // 2 - 1),
                        )
                    o_sb = outp.tile([128, NREP * HD], F16, tag="osb")
                    nc.scalar.copy(o_sb, o_list[tt])
                    r0 = CH * LAST + 128 * tt
                    nc.sync.dma_start(out=out[r0:r0 + 128, :], in_=o_sb)

            xh = x_first
            pres = {}
            for qc in range(NCHUNK):
                xh_next = emit_x_load(qc + 1) if qc + 1 < NCHUNK else None
                qT_sb = qkv_phase(qc, xh)
                xh = xh_next
                # wo for chunk qc-2: its AllGather finished a whole chunk ago
                if qc >= 2:
                    wo_compute(qc - 2, pres.pop(qc - 2))
                if qc >= 1:
                    pres[qc - 1] = wo_start(qc - 1)
                att_phase(qc, qT_sb)
            wo_compute(NCHUNK - 2, pres.pop(NCHUNK - 2))
            wo_last()

    nc.compile()
    _cached["nc"] = nc
    return nc


def _build_in_maps(inputs):
    return _shard_inputs(**inputs)


def _shard_inputs(x, cos, sin, wq, wk, wv, wo, start_pos):
    x = np.asarray(x, dtype=np.float32)
    cos = np.asarray(cos, dtype=np.float32)
    sin = np.asarray(sin, dtype=np.float32)
    wq = np.asarray(wq, dtype=np.float32)
    wk = np.asarray(wk, dtype=np.float32)
    wv = np.asarray(wv, dtype=np.float32)
    wo = np.asarray(wo, dtype=np.float32)
    sp = int(start_pos)

    xT = x[0].T  # (DIM, T)
    # per (chunk, half): [128, 16*512] partition-major contiguous
    xt = np.ascontiguousarray(
        xT.reshape(2, 16, 128, NCHUNK, CH).transpose(3, 0, 2, 1, 4).reshape(
            NCHUNK, 2, 128, 16 * CH
        )
    ).astype(np.float16)
    cosT = cos[sp:sp + T].T  # (64, T)
    sinT = sin[sp:sp + T].T
    cos2 = np.concatenate([cosT, cosT], axis=0).astype(np.float16)  # (128, T)
    sin2 = np.concatenate([-sinT, sinT], axis=0).astype(np.float16)

    kk = np.arange(128)[:, None]
    qq = np.arange(CH)[None, :]
    masks = np.concatenate(
        [(kk + 128 * d <= qq).astype(np.float32) for d in range(4)], axis=1
    ).astype(BF16NP)  # (128, 2048)

    def wgroup(wmat):  # (DIM, 128) -> (128, NKT*128), partition-major
        return wmat.reshape(NKT, 128, 128).transpose(1, 0, 2).reshape(
            128, NKT * 128
        )

    in_maps = []
    for c in range(N_CORES):
        qrows = slice(NREP * HD * c, NREP * HD * (c + 1))
        krows = slice(HD * c, HD * (c + 1))
        wqc = wq[qrows, :].T  # (DIM, 512)
        groups = [wv[krows, :].T] + [
            wqc[:, 128 * h:128 * (h + 1)] for h in range(NREP)
        ] + [wk[krows, :].T]
        wqkv = np.stack([wgroup(g) for g in groups]).astype(np.float16)
        in_maps.append({
            "xt": xt,
            "cos2": cos2,
            "sin2": sin2,
            "masks": masks,
            "wqkv": np.ascontiguousarray(wqkv),
            "woT": np.ascontiguousarray(wo[qrows, :].T).astype(np.float16),
        })
    return in_maps


def kernel(x, cos, sin, wq, wk, wv, wo, start_pos):
    in_maps = _shard_inputs(x, cos, sin, wq, wk, wv, wo, start_pos)
    nc = _build_kernel()
    res = run_bass_kernel_spmd(nc, in_maps, core_ids=list(range(N_CORES)))
    out = np.concatenate(
        [res.results[c]["out"].astype(np.float32) for c in range(N_CORES)],
        axis=1,
    )
    return out.reshape(1, T, DIM)
